# revision 10
# baseline (speedup 1.0000x reference)
"""Trainium2 Bass kernel for nn_Attention_49349174231422.

B=64,S=256,DIM=512,H=16,DH=32,W=256. Batch-sharded across 8 NeuronCores.

Per-core plan (8 batches), all matmuls float32r (TF32-class, 1cyc/row):
  loop0 (per b): hs -> PE-transpose -> hsT_all; V projection -> v_all with a
    ones column appended per head (33-wide slots).
  loopB (per head): bias context = biasT_h @ v, batched across all 8 batches
    in one matmul group (bias matrix is batch-independent) -> cb_sb.
  loop1 (per b): Q,K projections with host-permuted weight columns
    ([even-pair dims(16); odd-pair dims(16)] per head) -> RoPE via cos/sin
    tables + a PE swap-permutation matmul -> per-head transposed scores
    (2-way row-packed K=32) -> exp on ScalarE (scale folded) -> PV with the
    augmented v (sums ride along as a 33rd row) -> PE transpose back to
    [q, d] (sums become per-partition columns) -> out = ctx_exp * (1/sum)
    + bias context -> DMA.
"""
import sys

sys.path.insert(0, "/opt/trn_rl_repo")

import numpy as np

B, S, DIM = 64, 256, 512
H, DH, W = 16, 32, 256
NCORES = 8
BPC = B // NCORES
ROPE_BASE = 10000.0

_cache = {}


def _split_excess_waits(nc, max_waits=1):
    """walrus here rejects >1 sync-wait per instruction; spill extras onto
    engine-local NoOps placed immediately before the instruction."""
    from concourse import mybir

    ctr = 0
    for function in nc.m.functions:
        for block in function.blocks:
            insts = list(block.instructions)
            out = []
            changed = False
            for inst in insts:
                si = inst.sync_info
                if si is not None and si.on_wait and len(si.on_wait) > max_waits:
                    waits = list(si.on_wait)
                    spill, keep = waits[:-max_waits], waits[-max_waits:]
                    for w in spill:
                        ctr += 1
                        out.append(
                            mybir.InstNoOp(
                                name=f"syncnop-{id(nc)}-{ctr}",
                                sync_info=mybir.SyncInfo(on_wait=[w], on_update=[]),
                                bass_nofuse=True,
                                engine=inst.engine,
                            )
                        )
                    inst.sync_info = mybir.SyncInfo(
                        on_wait=keep, on_update=list(si.on_update)
                    )
                    changed = True
                out.append(inst)
            if changed:
                block.instructions = out
    return nc


def _build():
    from concourse import bass, tile, mybir

    F32R = mybir.dt.float32r
    F32 = mybir.dt.float32
    EXP = mybir.ActivationFunctionType.Exp

    nc = bass.Bass(target_bir_lowering=False, trn_type="TRN2")

    hs_d = nc.dram_tensor("hs", [BPC, S, DIM], F32R, kind="ExternalInput")
    wq_d = nc.dram_tensor("wq", [DIM, DIM], F32R, kind="ExternalInput")
    wk_d = nc.dram_tensor("wk", [DIM, DIM], F32R, kind="ExternalInput")
    wv_d = nc.dram_tensor("wv", [DIM, DIM], F32R, kind="ExternalInput")
    cos_d = nc.dram_tensor("cosm", [128, S], F32, kind="ExternalInput")
    sinp_d = nc.dram_tensor("sinp", [128, S], F32, kind="ExternalInput")
    sw_d = nc.dram_tensor("swp", [128, 128], F32R, kind="ExternalInput")
    id_d = nc.dram_tensor("idm", [128, 128], F32R, kind="ExternalInput")
    ones_d = nc.dram_tensor("onesv", [128, 1], F32R, kind="ExternalInput")
    # biasT[kp, h, kc, q] = bias_table[q - (kc*128+kp) + W-1, h]
    bt_d = nc.dram_tensor("biasT", [128, H, 2, S], F32R, kind="ExternalInput")
    out_d = nc.dram_tensor("out", [BPC, S, DIM], F32, kind="ExternalOutput")

    SCL = 1.0 / float(np.sqrt(DH))

    with tile.TileContext(nc) as tc:
        with (
            tc.tile_pool(name="const", bufs=1) as cp,
            tc.tile_pool(name="state", bufs=1) as st,
            tc.tile_pool(name="ps", bufs=1, space="PSUM") as ps,
        ):
            # ---------- constants ----------
            wq_sb = cp.tile([128, 4, DIM], F32R, name="wq_sb")
            wk_sb = cp.tile([128, 4, DIM], F32R, name="wk_sb")
            wv_sb = cp.tile([128, 4, DIM], F32R, name="wv_sb")
            for c in range(4):
                nc.sync.dma_start(wq_sb[:, c, :], wq_d[c * 128:(c + 1) * 128, :])
                nc.sync.dma_start(wk_sb[:, c, :], wk_d[c * 128:(c + 1) * 128, :])
                nc.sync.dma_start(wv_sb[:, c, :], wv_d[c * 128:(c + 1) * 128, :])
            cos_sb = cp.tile([128, S], F32, name="cos_sb")
            sinp_sb = cp.tile([128, S], F32, name="sinp_sb")
            sw_sb = cp.tile([128, 128], F32R, name="sw_sb")
            id_sb = cp.tile([128, 128], F32R, name="id_sb")
            ones_sb = cp.tile([128, 1], F32R, name="ones_sb")
            nc.sync.dma_start(cos_sb[:], cos_d[:])
            nc.sync.dma_start(sinp_sb[:], sinp_d[:])
            nc.sync.dma_start(sw_sb[:], sw_d[:])
            nc.sync.dma_start(id_sb[:], id_d[:])
            nc.sync.dma_start(ones_sb[:], ones_d[:])
            # persistent per-core state
            hsT_all = st.tile([128, 4, BPC, S], F32R, name="hsT_all")
            v_all = st.tile([128, 2, BPC, H, 33], F32R, name="v_all")
            cb_sb = st.tile([128, 2, H, BPC * 32], F32, name="cb_sb")
            # ones column of v_all (33rd col of every head slot)
            for kc in range(2):
                nc.vector.tensor_copy(
                    v_all[:, kc, :, :, 32:33],
                    ones_sb[:].rearrange("p (a b c) -> p a b c", b=1, c=1)
                    .to_broadcast((128, BPC, H, 1)),
                )

            # ---------- loop0 + loopB in a scoped pool (biasT freed after) --
            bp = tc.alloc_tile_pool(name="biasp", bufs=1)
            ld = tc.alloc_tile_pool(name="ld", bufs=2)
            bt_sb = bp.tile([128, H, 2, S], F32R, name="bt_sb")
            for h in range(H):
                nc.sync.dma_start(bt_sb[:, h, :, :], bt_d[:, h, :, :])
            for b in range(BPC):
                hs_sb = ld.tile([128, 2, DIM], F32R, name="hs_sb")
                for sc in range(2):
                    nc.sync.dma_start(hs_sb[:, sc, :],
                                      hs_d[b, sc * 128:(sc + 1) * 128, :])
                for c in range(4):
                    pT = ps.tile([128, 512], F32, name="pT",
                                 tag=f"w{c % 3}")[:, 0:S]
                    for sc in range(2):
                        nc.tensor.transpose(
                            pT.bitcast(F32R)[:, sc * 128:(sc + 1) * 128],
                            hs_sb[:, sc, c * 128:(c + 1) * 128], id_sb[:],
                        )
                    nc.vector.tensor_copy(hsT_all[:, c, b, :], pT[:])
                for sc in range(2):
                    psV = ps.tile([128, 512], F32, name="psV", tag=f"w{sc}")
                    for c in range(4):
                        nc.tensor.matmul(
                            psV[:], hsT_all[:, c, b, sc * 128:(sc + 1) * 128],
                            wv_sb[:, c, :], start=(c == 0), stop=(c == 3),
                        )
                    nc.vector.tensor_copy(
                        v_all[:, sc, b, :, 0:32],
                        psV[:].rearrange("p (a b) -> p a b", b=32))

            # ---------- loopB: batched bias context ----------
            for h in range(H):
                for qc in range(2):
                    cbp = ps.tile([128, 512], F32, name="cbp",
                                  tag=f"w{qc}")[:, 0:BPC * 32]
                    for kc in range(2):
                        nc.tensor.matmul(
                            cbp[:],
                            bt_sb[:, h, kc, qc * 128:(qc + 1) * 128],
                            v_all[:, kc, :, h, 0:32],
                            start=(kc == 0), stop=(kc == 1),
                        )
                    nc.vector.tensor_copy(cb_sb[:, qc, h, :], cbp[:])
            ld.release()
            bp.release()
            wp = tc.alloc_tile_pool(name="work", bufs=2)
            at = tc.alloc_tile_pool(name="att", bufs=2)

            # ---------- loop1: Q,K, rope, attention ----------
            for b in range(BPC):
                qT_sb = wp.tile([128, 4, S], F32R, name="qT_sb")
                kT_sb = wp.tile([128, 4, S], F32R, name="kT_sb")
                for (w_sb, oT_sb) in ((wq_sb, qT_sb), (wk_sb, kT_sb)):
                    for t in range(4):
                        psQ = ps.tile([128, 512], F32, name="psQ",
                                      tag=f"w{t % 2}")[:, 0:S]
                        for c in range(4):
                            nc.tensor.matmul(
                                psQ[:], w_sb[:, c, t * 128:(t + 1) * 128],
                                hsT_all[:, c, b, :], start=(c == 0),
                                stop=(c == 3),
                            )
                        x = wp.tile([128, S], F32R, name="x", tag="ropex")
                        nc.vector.tensor_copy(x[:], psQ[:])
                        xs = wp.tile([128, S], F32R, name="xs", tag="ropexs")
                        nc.vector.tensor_mul(xs[:], x.bitcast(F32)[:], sinp_sb[:])
                        psS = ps.tile([128, 512], F32, name="psS",
                                      tag="w2")[:, 0:S]
                        nc.tensor.matmul(psS[:], sw_sb[:], xs[:],
                                         start=True, stop=True)
                        t1 = wp.tile([128, S], F32, name="t1", tag="ropet1")
                        nc.vector.tensor_mul(t1[:], x.bitcast(F32)[:], cos_sb[:])
                        nc.vector.tensor_add(oT_sb[:, t, :], t1[:], psS[:])

                psOT = [[ps.tile([128, 8, 64], F32, name=f"psOT{qc}{hf}",
                                 tag=f"ot{qc}{hf}") for hf in range(2)]
                        for qc in range(2)]
                for g in range(4):
                    hf = g // 2
                    expT = [at.tile([128, 2, S], F32R, name=f"expT{j}",
                                    tag=f"expT{j}") for j in range(4)]
                    for kc in range(2):
                        for jj in range(2):  # 2-way row-packed bursts
                            psSC = [ps.tile([128, 512], F32, name=f"psSC{i}",
                                            tag=f"w{i}")[:, 0:S]
                                    for i in range(2)]
                            for i in range(2):
                                j = 2 * jj + i
                                nc.tensor.matmul(
                                    psSC[i][:],
                                    kT_sb[32 * j:32 * (j + 1), g,
                                          kc * 128:(kc + 1) * 128],
                                    qT_sb[32 * j:32 * (j + 1), g, :],
                                    start=True, stop=True,
                                    tile_position=(32 * j, 0),
                                    skip_group_check=True,
                                )
                            for i in range(2):
                                j = 2 * jj + i
                                nc.scalar.activation(expT[j][:, kc, :],
                                                     psSC[i][:], EXP, scale=SCL)
                    for j in range(4):
                        h = 4 * g + j
                        hh = (g % 2) * 4 + j  # head index within half
                        psCE = ps.tile([128, 512], F32, name="psCE",
                                       tag="ce")[:, 0:S]
                        for kc in range(2):
                            nc.tensor.matmul(
                                psCE[0:33, :],
                                v_all[:, kc, b, h, :],
                                expT[j][:, kc, :],
                                start=(kc == 0), stop=(kc == 1),
                            )
                        ce = at.tile([128, S], F32R, name="ce", tag="ce_sb")
                        nc.vector.tensor_copy(ce[0:33, :], psCE[0:33, :])
                        for qc in range(2):
                            nc.tensor.transpose(
                                psOT[qc][hf].bitcast(F32R)[:, hh, :],
                                ce[0:64, qc * 128:(qc + 1) * 128],
                                id_sb[0:64, 0:64],
                            )

                for qc in range(2):
                    o_sb = at.tile([128, 2, 256], F32, name="o_sb", tag="o_sb")
                    for hf in range(2):
                        rc = at.tile([128, 8], F32, name="rc", tag="rc")
                        nc.vector.reciprocal(rc[:], psOT[qc][hf][:, :, 32])
                        u = at.tile([128, 8, 32], F32, name="u", tag="u")
                        nc.vector.tensor_mul(
                            u[:],
                            psOT[qc][hf][:, :, 0:32],
                            rc[:].rearrange("p (a b) -> p a b", b=1)
                            .to_broadcast((128, 8, 32)),
                        )
                        nc.vector.tensor_add(
                            o_sb[:, hf, :].rearrange("p (a b) -> p a b", b=32),
                            u[:],
                            cb_sb[:, qc, 8 * hf:8 * (hf + 1),
                                  b * 32:(b + 1) * 32],
                        )
                        nc.sync.dma_start(
                            out_d[b, qc * 128:(qc + 1) * 128,
                                  hf * 256:(hf + 1) * 256],
                            o_sb[:, hf, :])

            at.release()
            wp.release()

    _split_excess_waits(nc)
    return nc


def _host_consts():
    p = np.arange(DIM)
    h = p // 32
    r = p % 32
    orig = np.where(r < 16, h * 32 + 2 * r, h * 32 + 2 * (r - 16) + 1)
    rows = np.arange(128)
    jj = rows % 16
    inv_freq = 1.0 / (ROPE_BASE ** (np.arange(0, DH, 2, dtype=np.float64) / DH))
    pos = np.arange(S, dtype=np.float64)
    ang = pos[None, :] * inv_freq[jj][:, None]
    cosm = np.cos(ang).astype(np.float32)
    sgn = np.where((rows % 32) < 16, 1.0, -1.0)[:, None]
    sinp = (np.sin(ang) * sgn).astype(np.float32)
    swp = np.zeros((128, 128), dtype=np.float32)
    swap_rows = (rows // 32) * 32 + ((rows % 32) + 16) % 32
    swp[swap_rows, rows] = 1.0
    return orig, cosm, sinp, swp


def kernel(hidden_states, Wq, bq, Wk, bk, Wv, bv, bias_table):
    hidden_states = np.ascontiguousarray(np.asarray(hidden_states, np.float32))
    Wq = np.asarray(Wq, np.float32)
    Wk = np.asarray(Wk, np.float32)
    Wv = np.asarray(Wv, np.float32)
    bias_table = np.asarray(bias_table, np.float32)
    assert not (np.any(bq) or np.any(bk) or np.any(bv)), \
        "nonzero qkv bias not supported by this kernel build"

    from concourse.bass_utils import run_bass_kernel_spmd

    if "nc" not in _cache:
        _cache["nc"] = _build()
    nc = _cache["nc"]

    orig, cosm, sinp, swp = _host_consts()
    idm = np.eye(128, dtype=np.float32)
    ones = np.ones((128, 1), dtype=np.float32)
    kp = np.arange(128)[:, None, None, None]
    hh = np.arange(H)[None, :, None, None]
    kcg = np.arange(2)[None, None, :, None]
    qq = np.arange(S)[None, None, None, :]
    idx = qq - (kcg * 128 + kp) + (W - 1)
    biasT = np.ascontiguousarray(bias_table[idx, hh].astype(np.float32))

    shared = {
        "wq": np.ascontiguousarray(Wq[:, orig]),
        "wk": np.ascontiguousarray(Wk[:, orig]),
        "wv": np.ascontiguousarray(Wv),
        "cosm": cosm, "sinp": sinp, "swp": swp, "idm": idm, "onesv": ones,
        "biasT": biasT,
    }
    in_maps = []
    for c in range(NCORES):
        m = dict(shared)
        m["hs"] = np.ascontiguousarray(hidden_states[c * BPC:(c + 1) * BPC])
        in_maps.append(m)

    res = run_bass_kernel_spmd(nc, in_maps, core_ids=list(range(NCORES)))
    out = np.concatenate([r["out"] for r in res.results], axis=0)
    return out.astype(np.float32)


if __name__ == "__main__":
    rng = np.random.default_rng(0)
    hs = rng.standard_normal((B, S, DIM), dtype=np.float32)
    w = rng.standard_normal((3, DIM, DIM), dtype=np.float32) / np.sqrt(DIM)
    bt = rng.standard_normal((2 * W - 1, H), dtype=np.float32) * 0.02
    z = np.zeros(DIM, np.float32)
    o = kernel(hs, w[0], z, w[1], z, w[2], z, bt)
    print("out", o.shape, o.dtype, np.abs(o).max())
